# revision 33
# baseline (speedup 1.0000x reference)
"""DomainBatchNorm Trainium2 kernel.

Math (per sample row r with one-hot domain mask m_r over D=8 domains):
    scale = gammas * rsqrt(pop_vars + eps)            # [D, F]
    shift = betas  - pop_means * scale                # [D, F]
    y[r]  = x[r] * (m_r @ scale) + (m_r @ shift)      # [B, F]

Strategy: pure data-parallel over the batch dim on 8 NeuronCores (4096 rows
per core).  Per 128-row tile, the [128, F] effective scale/shift are produced
on the TensorEngine as mask-tile @ table matmuls (K = D = 8).  The mask is
one-hot so it is exact in bf16; the fp32 scale/shift tables are fed through
the PE as a bf16 hi + bf16 lo split, accumulated in fp32 PSUM, which
reconstructs them to ~2^-18 relative accuracy.  The elementwise
y = x*es + et runs as two fp32 tensor_tensor ops on the VectorEngine.
The kernel is memory-roofline bound: 16 MiB in + 16 MiB out per core.
"""

import numpy as np
import ml_dtypes

B, F, D = 32768, 1024, 8
EPS = 1e-5
N_CORES = 8
ROWS = B // N_CORES          # 4096 rows per core
P = 128                      # partitions / rows per tile
N_TILES = ROWS // P          # 32
HALF = 512                   # fp32 matmul moving-operand max (one PSUM bank)

_NC_CACHE = {}


def _build_nc(reps=1, variant="full"):
    import concourse.bacc as bacc
    import concourse.tile as tile
    from concourse import mybir

    f32 = mybir.dt.float32
    bf16 = mybir.dt.bfloat16

    nc = bacc.Bacc(
        "TRN2", target_bir_lowering=False, debug=False, num_devices=N_CORES
    )

    f16 = mybir.dt.float16

    # scale/shift tables as a 2-term split: term0 fp16 (11 mantissa bits) +
    # term1 bf16 residual -> tables reconstruct in fp32 PSUM to ~2^-21 rel.
    # The one-hot mask is exact in both dtypes; each term's matmul uses a
    # mask copy matching its table dtype.
    TERM_DTS = [f16, bf16]

    x = nc.dram_tensor("x", [ROWS, F], f32, kind="ExternalInput").ap()
    maskT_h = nc.dram_tensor("maskT_h", [D, ROWS], f16, kind="ExternalInput").ap()
    maskT_b = nc.dram_tensor("maskT_b", [D, ROWS], bf16, kind="ExternalInput").ap()
    s_terms = [
        nc.dram_tensor(f"s{k}", [D, F], dt, kind="ExternalInput").ap()
        for k, dt in enumerate(TERM_DTS)
    ]
    t_terms = [
        nc.dram_tensor(f"t{k}", [D, F], dt, kind="ExternalInput").ap()
        for k, dt in enumerate(TERM_DTS)
    ]
    y = nc.dram_tensor("y", [ROWS, F], f32, kind="ExternalOutput").ap()

    # super-tile: SUP row-tiles of 128 rows move as ONE DMA (amortizes the
    # per-InstDMACopy fixed cost on the HWDGE ring); loads issue on the SP
    # ring, stores on the ACT ring so the two directions don't serialize on
    # one HWDGE FIFO.
    SUP = 1                      # row-tiles per super-tile
    store_eng = "scalar"
    BUFS = 8
    alt = False
    NTERMS = len(TERM_DTS)
    for part in variant.split("_"):
        if part.startswith("sup"):
            SUP = int(part[3:])
        if part in ("sp", "scalar", "gpsimd"):
            store_eng = part
        if part.startswith("b") and part[1:].isdigit():
            BUFS = int(part[1:])
        if part == "alt":
            alt = True
        if part == "uno":
            NTERMS = 1
    N_SUP = N_TILES // SUP

    with tile.TileContext(nc) as tc:
        with (
            tc.tile_pool(name="consts", bufs=1) as consts,
            tc.tile_pool(name="xp", bufs=BUFS) as xp,
            tc.tile_pool(name="tmpp", bufs=4) as tmpp,
            tc.tile_pool(name="outp", bufs=BUFS) as outp,
            tc.tile_pool(name="psp", bufs=2, space="PSUM") as psp,
            tc.tile_pool(name="ptp", bufs=2, space="PSUM") as ptp,
        ):
            # consts go via the gpsimd (SWDGE) ring so they don't sit ahead
            # of the first x-tile loads in the SP HWDGE FIFO
            mask_srcs = [maskT_h, maskT_b]
            m_sb, s_sb, t_sb = [], [], []
            for k in range(NTERMS):
                m_k = consts.tile([D, ROWS], TERM_DTS[k], tag=f"m{k}")
                nc.gpsimd.dma_start(out=m_k, in_=mask_srcs[k])
                m_sb.append(m_k)
                s_k = consts.tile([D, F], TERM_DTS[k], tag=f"s{k}")
                nc.gpsimd.dma_start(out=s_k, in_=s_terms[k])
                s_sb.append(s_k)
                t_k = consts.tile([D, F], TERM_DTS[k], tag=f"t{k}")
                nc.gpsimd.dma_start(out=t_k, in_=t_terms[k])
                t_sb.append(t_k)

            def body():
                for i in range(N_SUP):
                    r0 = i * SUP * P
                    load = nc.scalar if (alt and i % 2) else nc.sync
                    nc_store = nc.sync if (alt and i % 2) else None
                    if "storeonly" not in variant:
                        xt = xp.tile([P, SUP, F], f32)
                        load.dma_start(
                            out=xt,
                            in_=x[r0 : r0 + SUP * P, :].rearrange(
                                "(j p) f -> p j f", p=P
                            ),
                        )
                    if "loadonly" in variant:
                        continue
                    ot = outp.tile([P, SUP, F], f32)
                    if "storeonly" in variant:
                        nc.gpsimd.memset(ot, 0.0)
                    for j in range(SUP):
                        if "storeonly" in variant:
                            continue
                        if variant == "dma_copy":
                            nc.scalar.copy(ot[:, j, :], xt[:, j, :])
                            continue
                        cols = slice(r0 + j * P, r0 + (j + 1) * P)
                        ps = psp.tile([P, F], f32)  # eff_scale
                        pt = ptp.tile([P, F], f32)  # eff_shift
                        for h in (0, 1):
                            c = slice(h * HALF, (h + 1) * HALF)
                            for k in range(NTERMS):
                                nc.tensor.matmul(
                                    ps[:, c], lhsT=m_sb[k][:, cols], rhs=s_sb[k][:, c],
                                    start=(k == 0), stop=(k == NTERMS - 1),
                                )
                            for k in range(NTERMS):
                                nc.tensor.matmul(
                                    pt[:, c], lhsT=m_sb[k][:, cols], rhs=t_sb[k][:, c],
                                    start=(k == 0), stop=(k == NTERMS - 1),
                                )

                        tmp = tmpp.tile([P, F], f32)
                        nc.vector.tensor_mul(tmp, xt[:, j, :], ps)
                        nc.vector.tensor_add(ot[:, j, :], tmp, pt)

                    if "loadonly" in variant:
                        continue
                    store = {"scalar": nc.scalar, "sp": nc.sync, "gpsimd": nc.gpsimd}[
                        store_eng
                    ]
                    if nc_store is not None:
                        store = nc_store
                    store.dma_start(
                        out=y[r0 : r0 + SUP * P, :].rearrange("(j p) f -> p j f", p=P),
                        in_=ot,
                    )

            if reps == 1:
                body()
            else:
                # bench mode: repeat the whole pipeline in a HW loop so one
                # NEFF execution carries `reps` kernel-equivalents of work
                with tc.For_i(0, reps, 1):
                    body()

    nc.compile()
    return nc


def _get_nc(reps=1, variant="full"):
    key = (reps, variant)
    if key not in _NC_CACHE:
        _NC_CACHE[key] = _build_nc(reps, variant)
    return _NC_CACHE[key]


TERM_NP_DTS = [np.float16, ml_dtypes.bfloat16]


def _split_terms(v64):
    """Split a float64 array into fp16 + bf16 terms summing to ~v64
    (~2^-21 relative residual; fp16 range is safe for these tables)."""
    terms, rem = [], v64
    for dt in TERM_NP_DTS:
        t = rem.astype(dt)
        terms.append(t)
        rem = rem - t.astype(np.float64)
    return terms


def _prep_in_maps(inputs, mask, gammas, betas, pop_means, pop_vars):
    # Fold the per-domain params into scale/shift tables (tiny [D, F] work),
    # in float64 so the dtype splits capture the true value.
    scale64 = gammas.astype(np.float64) / np.sqrt(pop_vars.astype(np.float64) + EPS)
    shift64 = betas.astype(np.float64) - pop_means.astype(np.float64) * scale64
    s_terms = _split_terms(scale64)
    t_terms = _split_terms(shift64)

    maskT64 = np.ascontiguousarray(mask.astype(np.float64).T)

    in_maps = []
    for c in range(N_CORES):
        r0, r1 = c * ROWS, (c + 1) * ROWS
        m_sl = maskT64[:, r0:r1]
        im = {
            "x": np.ascontiguousarray(inputs[r0:r1]),
            # one-hot: exact in both fp16 and bf16
            "maskT_h": np.ascontiguousarray(m_sl.astype(np.float16)),
            "maskT_b": np.ascontiguousarray(m_sl.astype(ml_dtypes.bfloat16)),
        }
        for k in range(len(TERM_NP_DTS)):
            im[f"s{k}"] = s_terms[k]
            im[f"t{k}"] = t_terms[k]
        in_maps.append(im)
    return in_maps


def kernel(inputs, mask, gammas, betas, pop_means, pop_vars, _trace=False, **_tr_kw):
    from concourse.bass_utils import run_bass_kernel_spmd

    in_maps = _prep_in_maps(inputs, mask, gammas, betas, pop_means, pop_vars)
    nc = _get_nc()
    res = run_bass_kernel_spmd(
        nc, in_maps, list(range(N_CORES)), trace=_trace, **_tr_kw
    )
    out = np.concatenate([res.results[c]["y"] for c in range(N_CORES)], axis=0)
    if _trace:
        kernel.last_results = res
    return out


# revision 39
# speedup vs baseline: 1.3905x; 1.3905x over previous
"""DomainBatchNorm Trainium2 kernel.

Math (per sample row r with one-hot domain mask m_r over D=8 domains):
    scale = gammas * rsqrt(pop_vars + eps)            # [D, F]
    shift = betas  - pop_means * scale                # [D, F]
    y[r]  = x[r] * (m_r @ scale) + (m_r @ shift)      # [B, F]

Strategy: pure data-parallel over the batch dim on 8 NeuronCores (4096 rows
per core).  Per 128-row tile, the [128, F] effective scale/shift are produced
on the TensorEngine as mask-tile @ table matmuls (K = D = 8).  The mask is
one-hot so it is exact in bf16; the fp32 scale/shift tables are fed through
the PE as a bf16 hi + bf16 lo split, accumulated in fp32 PSUM, which
reconstructs them to ~2^-18 relative accuracy.  The elementwise
y = x*es + et runs as two fp32 tensor_tensor ops on the VectorEngine.
The kernel is memory-roofline bound: 16 MiB in + 16 MiB out per core.
"""

import numpy as np
import ml_dtypes

B, F, D = 32768, 1024, 8
EPS = 1e-5
N_CORES = 8
ROWS = B // N_CORES          # 4096 rows per core
P = 128                      # partitions / rows per tile
N_TILES = ROWS // P          # 32
HALF = 512                   # fp32 matmul moving-operand max (one PSUM bank)

_NC_CACHE = {}


def _build_nc(reps=1, variant="full"):
    import concourse.bacc as bacc
    import concourse.tile as tile
    from concourse import mybir

    f32 = mybir.dt.float32
    bf16 = mybir.dt.bfloat16

    nc = bacc.Bacc(
        "TRN2", target_bir_lowering=False, debug=False, num_devices=N_CORES
    )

    # scale/shift tables as a hi+lo bf16 split accumulated in fp32 PSUM:
    # tables reconstruct to ~2^-18 rel. (An fp16 hi term would reach ~2^-21
    # but fp16 matmuls measure ~40% slower on TRN2 HW, and a 3rd bf16 term
    # makes the cold-clocked PE the critical path — bf16 hi+lo is the sweet
    # spot.) The one-hot mask is exact in bf16.
    NTERMS = 2

    x = nc.dram_tensor("x", [ROWS, F], f32, kind="ExternalInput").ap()
    maskT = nc.dram_tensor("maskT", [D, ROWS], bf16, kind="ExternalInput").ap()
    s_terms = [
        nc.dram_tensor(f"s{k}", [D, F], bf16, kind="ExternalInput").ap()
        for k in range(NTERMS)
    ]
    t_terms = [
        nc.dram_tensor(f"t{k}", [D, F], bf16, kind="ExternalInput").ap()
        for k in range(NTERMS)
    ]
    y = nc.dram_tensor("y", [ROWS, F], f32, kind="ExternalOutput").ap()

    # super-tile: SUP row-tiles of 128 rows move as ONE DMA (amortizes the
    # per-InstDMACopy fixed cost on the HWDGE ring); loads issue on the SP
    # ring, stores on the ACT ring so the two directions don't serialize on
    # one HWDGE FIFO.
    SUP = 1                      # row-tiles per super-tile
    store_eng = "scalar"
    BUFS = 8
    alt = False
    for part in variant.split("_"):
        if part.startswith("sup"):
            SUP = int(part[3:])
        if part in ("sp", "scalar", "gpsimd"):
            store_eng = part
        if part.startswith("b") and part[1:].isdigit():
            BUFS = int(part[1:])
        if part == "alt":
            alt = True
    N_SUP = N_TILES // SUP

    with tile.TileContext(nc) as tc:
        with (
            tc.tile_pool(name="consts", bufs=1) as consts,
            tc.tile_pool(name="xp", bufs=BUFS) as xp,
            tc.tile_pool(name="tmpp", bufs=4) as tmpp,
            tc.tile_pool(name="outp", bufs=BUFS) as outp,
            tc.tile_pool(name="psp", bufs=2, space="PSUM") as psp,
            tc.tile_pool(name="ptp", bufs=2, space="PSUM") as ptp,
        ):
            # consts go via the gpsimd (SWDGE) ring so they don't sit ahead
            # of the first x-tile loads in the SP HWDGE FIFO
            mT = consts.tile([D, ROWS], bf16)
            nc.gpsimd.dma_start(out=mT, in_=maskT)
            s_sb, t_sb = [], []
            for k in range(NTERMS):
                s_k = consts.tile([D, F], bf16, tag=f"s{k}")
                nc.gpsimd.dma_start(out=s_k, in_=s_terms[k])
                s_sb.append(s_k)
                t_k = consts.tile([D, F], bf16, tag=f"t{k}")
                nc.gpsimd.dma_start(out=t_k, in_=t_terms[k])
                t_sb.append(t_k)

            def body():
                for i in range(N_SUP):
                    r0 = i * SUP * P
                    load = nc.scalar if (alt and i % 2) else nc.sync
                    nc_store = nc.sync if (alt and i % 2) else None
                    if "storeonly" not in variant:
                        xt = xp.tile([P, SUP, F], f32)
                        load.dma_start(
                            out=xt,
                            in_=x[r0 : r0 + SUP * P, :].rearrange(
                                "(j p) f -> p j f", p=P
                            ),
                        )
                    if "loadonly" in variant:
                        continue
                    ot = outp.tile([P, SUP, F], f32)
                    if "storeonly" in variant:
                        nc.gpsimd.memset(ot, 0.0)
                    for j in range(SUP):
                        if "storeonly" in variant:
                            continue
                        if variant == "dma_copy":
                            nc.scalar.copy(ot[:, j, :], xt[:, j, :])
                            continue
                        w = mT[:, r0 + j * P : r0 + (j + 1) * P]  # [D, P] lhsT
                        ps = psp.tile([P, F], f32)  # eff_scale
                        pt = ptp.tile([P, F], f32)  # eff_shift
                        for h in (0, 1):
                            c = slice(h * HALF, (h + 1) * HALF)
                            for k in range(NTERMS):
                                nc.tensor.matmul(
                                    ps[:, c], lhsT=w, rhs=s_sb[k][:, c],
                                    start=(k == 0), stop=(k == NTERMS - 1),
                                )
                            for k in range(NTERMS):
                                nc.tensor.matmul(
                                    pt[:, c], lhsT=w, rhs=t_sb[k][:, c],
                                    start=(k == 0), stop=(k == NTERMS - 1),
                                )

                        tmp = tmpp.tile([P, F], f32)
                        nc.vector.tensor_mul(tmp, xt[:, j, :], ps)
                        nc.vector.tensor_add(ot[:, j, :], tmp, pt)

                    if "loadonly" in variant:
                        continue
                    store = {"scalar": nc.scalar, "sp": nc.sync, "gpsimd": nc.gpsimd}[
                        store_eng
                    ]
                    if nc_store is not None:
                        store = nc_store
                    store.dma_start(
                        out=y[r0 : r0 + SUP * P, :].rearrange("(j p) f -> p j f", p=P),
                        in_=ot,
                    )

            if reps == 1:
                body()
            else:
                # bench mode: repeat the whole pipeline in a HW loop so one
                # NEFF execution carries `reps` kernel-equivalents of work
                with tc.For_i(0, reps, 1):
                    body()

    nc.compile()
    return nc


def _get_nc(reps=1, variant="full"):
    key = (reps, variant)
    if key not in _NC_CACHE:
        _NC_CACHE[key] = _build_nc(reps, variant)
    return _NC_CACHE[key]


def _split_terms(v64, nterms=2):
    """Split a float64 array into bf16 hi/lo terms summing to ~v64
    (~2^-18 relative residual)."""
    bf = ml_dtypes.bfloat16
    terms, rem = [], v64
    for _ in range(nterms):
        t = rem.astype(bf)
        terms.append(t)
        rem = rem - t.astype(np.float64)
    return terms


def _prep_in_maps(inputs, mask, gammas, betas, pop_means, pop_vars):
    # Fold the per-domain params into scale/shift tables (tiny [D, F] work),
    # in float64 so the bf16 splits capture the true value.
    scale64 = gammas.astype(np.float64) / np.sqrt(pop_vars.astype(np.float64) + EPS)
    shift64 = betas.astype(np.float64) - pop_means.astype(np.float64) * scale64
    s_terms = _split_terms(scale64)
    t_terms = _split_terms(shift64)

    # one-hot mask: exact in bf16
    maskT = np.ascontiguousarray(mask.astype(ml_dtypes.bfloat16).T)

    in_maps = []
    for c in range(N_CORES):
        r0, r1 = c * ROWS, (c + 1) * ROWS
        im = {
            "x": np.ascontiguousarray(inputs[r0:r1]),
            "maskT": np.ascontiguousarray(maskT[:, r0:r1]),
        }
        for k in range(2):
            im[f"s{k}"] = s_terms[k]
            im[f"t{k}"] = t_terms[k]
        in_maps.append(im)
    return in_maps


def kernel(inputs, mask, gammas, betas, pop_means, pop_vars, _trace=False, **_tr_kw):
    from concourse.bass_utils import run_bass_kernel_spmd

    in_maps = _prep_in_maps(inputs, mask, gammas, betas, pop_means, pop_vars)
    nc = _get_nc()
    res = run_bass_kernel_spmd(
        nc, in_maps, list(range(N_CORES)), trace=_trace, **_tr_kw
    )
    out = np.concatenate([res.results[c]["y"] for c in range(N_CORES)], axis=0)
    if _trace:
        kernel.last_results = res
    return out


# revision 42
# speedup vs baseline: 1.4050x; 1.0104x over previous
"""DomainBatchNorm Trainium2 kernel.

Math (per sample row r with one-hot domain mask m_r over D=8 domains):
    scale = gammas * rsqrt(pop_vars + eps)            # [D, F]
    shift = betas  - pop_means * scale                # [D, F]
    y[r]  = x[r] * (m_r @ scale) + (m_r @ shift)      # [B, F]

Strategy: pure data-parallel over the batch dim on 8 NeuronCores (4096 rows
per core, no communication).  Per 128-row tile, the [128, F] effective
scale/shift are produced on the TensorEngine as mask-tile @ table matmuls
(K = D = 8).  The mask is one-hot so it is exact in bf16; the fp32
scale/shift tables are fed through the PE as a bf16 hi + bf16 lo split,
accumulated in fp32 PSUM, reconstructing them to ~2^-18 relative accuracy
(overall output error ~5e-6 rel-to-max).  The elementwise y = x*es + et
runs as two fp32 tensor_tensor ops on the VectorEngine.

The kernel is HBM-bandwidth bound: 16 MiB in + 16 MiB out per core.
Measured on HW (8 cores concurrent): read-only ~54 us, write-only ~53 us,
full kernel ~114 us per core -- i.e. reads+writes share a ~315 GB/s
per-core HBM budget and the kernel sits at ~94% of that roofline, with all
PE/DVE compute hidden behind the DMA stream (a DMA-only variant measures
the same ~115 us).  x-tile loads issue on the SP HWDGE ring and y-tile
stores on the ACT HWDGE ring: a single ring executes its transfers FIFO,
and one 512 KiB transfer costs ~0.6 us fixed + ~1.6 us stream, so putting
all 64 transfers on one ring serializes to ~140 us while two rings keep
the 16 SDMA engines saturated.
"""

import sys

import numpy as np
import ml_dtypes

for _p in ("/opt/trn_rl_repo", "/opt/pypackages"):
    if _p not in sys.path:
        sys.path.append(_p)

B, F, D = 32768, 1024, 8
EPS = 1e-5
N_CORES = 8
ROWS = B // N_CORES          # 4096 rows per core
P = 128                      # partitions / rows per tile
N_TILES = ROWS // P          # 32
HALF = 512                   # fp32 matmul moving-operand max (one PSUM bank)

_NC_CACHE = {}


def _build_nc(reps=1, variant="full"):
    import concourse.bacc as bacc
    import concourse.tile as tile
    from concourse import mybir

    f32 = mybir.dt.float32
    bf16 = mybir.dt.bfloat16

    nc = bacc.Bacc(
        "TRN2", target_bir_lowering=False, debug=False, num_devices=N_CORES
    )

    # scale/shift tables as a hi+lo bf16 split accumulated in fp32 PSUM:
    # tables reconstruct to ~2^-18 rel. (An fp16 hi term would reach ~2^-21
    # but fp16 matmuls measure ~40% slower on TRN2 HW, and a 3rd bf16 term
    # makes the cold-clocked PE the critical path — bf16 hi+lo is the sweet
    # spot.) The one-hot mask is exact in bf16.
    NTERMS = 2

    x = nc.dram_tensor("x", [ROWS, F], f32, kind="ExternalInput").ap()
    maskT = nc.dram_tensor("maskT", [D, ROWS], bf16, kind="ExternalInput").ap()
    s_terms = [
        nc.dram_tensor(f"s{k}", [D, F], bf16, kind="ExternalInput").ap()
        for k in range(NTERMS)
    ]
    t_terms = [
        nc.dram_tensor(f"t{k}", [D, F], bf16, kind="ExternalInput").ap()
        for k in range(NTERMS)
    ]
    y = nc.dram_tensor("y", [ROWS, F], f32, kind="ExternalOutput").ap()

    # super-tile: SUP row-tiles of 128 rows move as ONE DMA (amortizes the
    # per-InstDMACopy fixed cost on the HWDGE ring); loads issue on the SP
    # ring, stores on the ACT ring so the two directions don't serialize on
    # one HWDGE FIFO.
    SUP = 1                      # row-tiles per super-tile
    store_eng = "scalar"
    BUFS = 8
    alt = False
    for part in variant.split("_"):
        if part.startswith("sup"):
            SUP = int(part[3:])
        if part in ("sp", "scalar", "gpsimd"):
            store_eng = part
        if part.startswith("b") and part[1:].isdigit():
            BUFS = int(part[1:])
        if part == "alt":
            alt = True
    N_SUP = N_TILES // SUP

    with tile.TileContext(nc) as tc:
        with (
            tc.tile_pool(name="consts", bufs=1) as consts,
            tc.tile_pool(name="xp", bufs=BUFS) as xp,
            tc.tile_pool(name="tmpp", bufs=4) as tmpp,
            tc.tile_pool(name="outp", bufs=BUFS) as outp,
            tc.tile_pool(name="psp", bufs=2, space="PSUM") as psp,
            tc.tile_pool(name="ptp", bufs=2, space="PSUM") as ptp,
        ):
            # consts go via the gpsimd (SWDGE) ring so they don't sit ahead
            # of the first x-tile loads in the SP HWDGE FIFO
            mT = consts.tile([D, ROWS], bf16)
            nc.gpsimd.dma_start(out=mT, in_=maskT)
            s_sb, t_sb = [], []
            for k in range(NTERMS):
                s_k = consts.tile([D, F], bf16, tag=f"s{k}")
                nc.gpsimd.dma_start(out=s_k, in_=s_terms[k])
                s_sb.append(s_k)
                t_k = consts.tile([D, F], bf16, tag=f"t{k}")
                nc.gpsimd.dma_start(out=t_k, in_=t_terms[k])
                t_sb.append(t_k)

            def body():
                for i in range(N_SUP):
                    r0 = i * SUP * P
                    load = nc.scalar if (alt and i % 2) else nc.sync
                    nc_store = nc.sync if (alt and i % 2) else None
                    if "storeonly" not in variant:
                        xt = xp.tile([P, SUP, F], f32)
                        load.dma_start(
                            out=xt,
                            in_=x[r0 : r0 + SUP * P, :].rearrange(
                                "(j p) f -> p j f", p=P
                            ),
                        )
                    if "loadonly" in variant:
                        continue
                    ot = outp.tile([P, SUP, F], f32)
                    if "storeonly" in variant:
                        nc.gpsimd.memset(ot, 0.0)
                    for j in range(SUP):
                        if "storeonly" in variant:
                            continue
                        if variant == "dma_copy":
                            nc.scalar.copy(ot[:, j, :], xt[:, j, :])
                            continue
                        w = mT[:, r0 + j * P : r0 + (j + 1) * P]  # [D, P] lhsT
                        ps = psp.tile([P, F], f32)  # eff_scale
                        pt = ptp.tile([P, F], f32)  # eff_shift
                        for h in (0, 1):
                            c = slice(h * HALF, (h + 1) * HALF)
                            for k in range(NTERMS):
                                nc.tensor.matmul(
                                    ps[:, c], lhsT=w, rhs=s_sb[k][:, c],
                                    start=(k == 0), stop=(k == NTERMS - 1),
                                )
                            for k in range(NTERMS):
                                nc.tensor.matmul(
                                    pt[:, c], lhsT=w, rhs=t_sb[k][:, c],
                                    start=(k == 0), stop=(k == NTERMS - 1),
                                )

                        tmp = tmpp.tile([P, F], f32)
                        nc.vector.tensor_mul(tmp, xt[:, j, :], ps)
                        nc.vector.tensor_add(ot[:, j, :], tmp, pt)

                    if "loadonly" in variant:
                        continue
                    store = {"scalar": nc.scalar, "sp": nc.sync, "gpsimd": nc.gpsimd}[
                        store_eng
                    ]
                    if nc_store is not None:
                        store = nc_store
                    store.dma_start(
                        out=y[r0 : r0 + SUP * P, :].rearrange("(j p) f -> p j f", p=P),
                        in_=ot,
                    )

            if reps == 1:
                body()
            else:
                # bench mode: repeat the whole pipeline in a HW loop so one
                # NEFF execution carries `reps` kernel-equivalents of work
                with tc.For_i(0, reps, 1):
                    body()

    nc.compile()
    return nc


def _get_nc(reps=1, variant="full"):
    key = (reps, variant)
    if key not in _NC_CACHE:
        _NC_CACHE[key] = _build_nc(reps, variant)
    return _NC_CACHE[key]


def _split_terms(v64, nterms=2):
    """Split a float64 array into bf16 hi/lo terms summing to ~v64
    (~2^-18 relative residual)."""
    bf = ml_dtypes.bfloat16
    terms, rem = [], v64
    for _ in range(nterms):
        t = rem.astype(bf)
        terms.append(t)
        rem = rem - t.astype(np.float64)
    return terms


def _prep_in_maps(inputs, mask, gammas, betas, pop_means, pop_vars):
    # Fold the per-domain params into scale/shift tables (tiny [D, F] work),
    # in float64 so the bf16 splits capture the true value.
    scale64 = gammas.astype(np.float64) / np.sqrt(pop_vars.astype(np.float64) + EPS)
    shift64 = betas.astype(np.float64) - pop_means.astype(np.float64) * scale64
    s_terms = _split_terms(scale64)
    t_terms = _split_terms(shift64)

    # one-hot mask: exact in bf16
    maskT = np.ascontiguousarray(mask.astype(ml_dtypes.bfloat16).T)

    in_maps = []
    for c in range(N_CORES):
        r0, r1 = c * ROWS, (c + 1) * ROWS
        im = {
            "x": np.ascontiguousarray(inputs[r0:r1]),
            "maskT": np.ascontiguousarray(maskT[:, r0:r1]),
        }
        for k in range(2):
            im[f"s{k}"] = s_terms[k]
            im[f"t{k}"] = t_terms[k]
        in_maps.append(im)
    return in_maps


def kernel(inputs, mask, gammas, betas, pop_means, pop_vars, _trace=False, **_tr_kw):
    from concourse.bass_utils import run_bass_kernel_spmd

    inputs = np.asarray(inputs, dtype=np.float32)
    mask = np.asarray(mask, dtype=np.float32)
    gammas = np.asarray(gammas, dtype=np.float32)
    betas = np.asarray(betas, dtype=np.float32)
    pop_means = np.asarray(pop_means, dtype=np.float32)
    pop_vars = np.asarray(pop_vars, dtype=np.float32)

    in_maps = _prep_in_maps(inputs, mask, gammas, betas, pop_means, pop_vars)
    nc = _get_nc()
    res = run_bass_kernel_spmd(
        nc, in_maps, list(range(N_CORES)), trace=_trace, **_tr_kw
    )
    out = np.concatenate([res.results[c]["y"] for c in range(N_CORES)], axis=0)
    if _trace:
        kernel.last_results = res
    return out


# revision 43
# speedup vs baseline: 1.5183x; 1.0806x over previous
"""DomainBatchNorm Trainium2 kernel.

Math (per sample row r with one-hot domain mask m_r over D=8 domains):
    scale = gammas * rsqrt(pop_vars + eps)            # [D, F]
    shift = betas  - pop_means * scale                # [D, F]
    y[r]  = x[r] * (m_r @ scale) + (m_r @ shift)      # [B, F]

Strategy: pure data-parallel over the batch dim on 8 NeuronCores (4096 rows
per core, no communication).  Per 128-row tile, the [128, F] effective
scale/shift are produced on the TensorEngine as mask-tile @ table matmuls
(K = D = 8).  The mask is one-hot so it is exact in bf16; the fp32
scale/shift tables are fed through the PE as a bf16 hi + bf16 lo split,
accumulated in fp32 PSUM, reconstructing them to ~2^-18 relative accuracy
(overall output error ~5e-6 rel-to-max).  The elementwise y = x*es + et
runs as two fp32 tensor_tensor ops on the VectorEngine.

The kernel is HBM-bandwidth bound: 16 MiB in + 16 MiB out per core.
Measured on HW (8 cores concurrent): read-only ~54 us, write-only ~53 us,
full kernel ~114 us per core -- i.e. reads+writes share a ~315 GB/s
per-core HBM budget and the kernel sits at ~94% of that roofline, with all
PE/DVE compute hidden behind the DMA stream (a DMA-only variant measures
the same ~115 us).  x-tile loads issue on the SP HWDGE ring and y-tile
stores on the ACT HWDGE ring: a single ring executes its transfers FIFO,
and one 512 KiB transfer costs ~0.6 us fixed + ~1.6 us stream, so putting
all 64 transfers on one ring serializes to ~140 us while two rings keep
the 16 SDMA engines saturated.
"""

import sys

import numpy as np
import ml_dtypes

for _p in ("/opt/trn_rl_repo", "/opt/pypackages"):
    if _p not in sys.path:
        sys.path.append(_p)

B, F, D = 32768, 1024, 8
EPS = 1e-5
N_CORES = 8
ROWS = B // N_CORES          # 4096 rows per core
P = 128                      # partitions / rows per tile
N_TILES = ROWS // P          # 32
HALF = 512                   # fp32 matmul moving-operand max (one PSUM bank)

_NC_CACHE = {}


def _build_nc(reps=1, variant="full"):
    import concourse.bacc as bacc
    import concourse.tile as tile
    from concourse import mybir

    f32 = mybir.dt.float32
    bf16 = mybir.dt.bfloat16

    nc = bacc.Bacc(
        "TRN2", target_bir_lowering=False, debug=False, num_devices=N_CORES
    )

    # scale/shift tables as a hi+lo bf16 split accumulated in fp32 PSUM:
    # tables reconstruct to ~2^-18 rel. (An fp16 hi term would reach ~2^-21
    # but fp16 matmuls measure ~40% slower on TRN2 HW, and a 3rd bf16 term
    # makes the cold-clocked PE the critical path — bf16 hi+lo is the sweet
    # spot.) The one-hot mask is exact in bf16.
    NTERMS = 1 if "uno" in variant else 2

    x = nc.dram_tensor("x", [ROWS, F], f32, kind="ExternalInput").ap()
    maskT = nc.dram_tensor("maskT", [D, ROWS], bf16, kind="ExternalInput").ap()
    s_terms = [
        nc.dram_tensor(f"s{k}", [D, F], bf16, kind="ExternalInput").ap()
        for k in range(NTERMS)
    ]
    t_terms = [
        nc.dram_tensor(f"t{k}", [D, F], bf16, kind="ExternalInput").ap()
        for k in range(NTERMS)
    ]
    y = nc.dram_tensor("y", [ROWS, F], f32, kind="ExternalOutput").ap()

    # super-tile: SUP row-tiles of 128 rows move as ONE DMA (amortizes the
    # per-InstDMACopy fixed cost on the HWDGE ring); loads issue on the SP
    # ring, stores on the ACT ring so the two directions don't serialize on
    # one HWDGE FIFO.
    SUP = 1                      # row-tiles per super-tile
    store_eng = "scalar"
    BUFS = 8
    alt = False
    for part in variant.split("_"):
        if part.startswith("sup"):
            SUP = int(part[3:])
        if part in ("sp", "scalar", "gpsimd"):
            store_eng = part
        if part.startswith("b") and part[1:].isdigit():
            BUFS = int(part[1:])
        if part == "alt":
            alt = True
    N_SUP = N_TILES // SUP

    with tile.TileContext(nc) as tc:
        with (
            tc.tile_pool(name="consts", bufs=1) as consts,
            tc.tile_pool(name="xp", bufs=BUFS) as xp,
            tc.tile_pool(name="tmpp", bufs=4) as tmpp,
            tc.tile_pool(name="outp", bufs=BUFS) as outp,
            tc.tile_pool(name="psp", bufs=2, space="PSUM") as psp,
            tc.tile_pool(name="ptp", bufs=2, space="PSUM") as ptp,
        ):
            # consts go via the gpsimd (SWDGE) ring so they don't sit ahead
            # of the first x-tile loads in the SP HWDGE FIFO
            mT = consts.tile([D, ROWS], bf16)
            nc.gpsimd.dma_start(out=mT, in_=maskT)
            s_sb, t_sb = [], []
            for k in range(NTERMS):
                s_k = consts.tile([D, F], bf16, tag=f"s{k}")
                nc.gpsimd.dma_start(out=s_k, in_=s_terms[k])
                s_sb.append(s_k)
                t_k = consts.tile([D, F], bf16, tag=f"t{k}")
                nc.gpsimd.dma_start(out=t_k, in_=t_terms[k])
                t_sb.append(t_k)

            def body():
                for i in range(N_SUP):
                    r0 = i * SUP * P
                    load = nc.scalar if (alt and i % 2) else nc.sync
                    nc_store = nc.sync if (alt and i % 2) else None
                    if "storeonly" not in variant:
                        xt = xp.tile([P, SUP, F], f32)
                        load.dma_start(
                            out=xt,
                            in_=x[r0 : r0 + SUP * P, :].rearrange(
                                "(j p) f -> p j f", p=P
                            ),
                        )
                    if "loadonly" in variant:
                        continue
                    ot = outp.tile([P, SUP, F], f32)
                    if "storeonly" in variant:
                        nc.gpsimd.memset(ot, 0.0)
                    for j in range(SUP):
                        if "storeonly" in variant:
                            continue
                        if variant == "dma_copy":
                            nc.scalar.copy(ot[:, j, :], xt[:, j, :])
                            continue
                        w = mT[:, r0 + j * P : r0 + (j + 1) * P]  # [D, P] lhsT
                        ps = psp.tile([P, F], f32)  # eff_scale
                        pt = ptp.tile([P, F], f32)  # eff_shift
                        for h in (0, 1):
                            c = slice(h * HALF, (h + 1) * HALF)
                            for k in range(NTERMS):
                                nc.tensor.matmul(
                                    ps[:, c], lhsT=w, rhs=s_sb[k][:, c],
                                    start=(k == 0), stop=(k == NTERMS - 1),
                                )
                            for k in range(NTERMS):
                                nc.tensor.matmul(
                                    pt[:, c], lhsT=w, rhs=t_sb[k][:, c],
                                    start=(k == 0), stop=(k == NTERMS - 1),
                                )

                        tmp = tmpp.tile([P, F], f32)
                        nc.vector.tensor_mul(tmp, xt[:, j, :], ps)
                        nc.vector.tensor_add(ot[:, j, :], tmp, pt)

                    if "loadonly" in variant:
                        continue
                    store = {"scalar": nc.scalar, "sp": nc.sync, "gpsimd": nc.gpsimd}[
                        store_eng
                    ]
                    if nc_store is not None:
                        store = nc_store
                    store.dma_start(
                        out=y[r0 : r0 + SUP * P, :].rearrange("(j p) f -> p j f", p=P),
                        in_=ot,
                    )

            if reps == 1:
                body()
            else:
                # bench mode: repeat the whole pipeline in a HW loop so one
                # NEFF execution carries `reps` kernel-equivalents of work
                with tc.For_i(0, reps, 1):
                    body()

    nc.compile()
    return nc


def _get_nc(reps=1, variant="full"):
    key = (reps, variant)
    if key not in _NC_CACHE:
        _NC_CACHE[key] = _build_nc(reps, variant)
    return _NC_CACHE[key]


def _split_terms(v64, nterms=2):
    """Split a float64 array into bf16 hi/lo terms summing to ~v64
    (~2^-18 relative residual)."""
    bf = ml_dtypes.bfloat16
    terms, rem = [], v64
    for _ in range(nterms):
        t = rem.astype(bf)
        terms.append(t)
        rem = rem - t.astype(np.float64)
    return terms


def _prep_in_maps(inputs, mask, gammas, betas, pop_means, pop_vars):
    # Fold the per-domain params into scale/shift tables (tiny [D, F] work),
    # in float64 so the bf16 splits capture the true value.
    scale64 = gammas.astype(np.float64) / np.sqrt(pop_vars.astype(np.float64) + EPS)
    shift64 = betas.astype(np.float64) - pop_means.astype(np.float64) * scale64
    s_terms = _split_terms(scale64)
    t_terms = _split_terms(shift64)

    # one-hot mask: exact in bf16
    maskT = np.ascontiguousarray(mask.astype(ml_dtypes.bfloat16).T)

    in_maps = []
    for c in range(N_CORES):
        r0, r1 = c * ROWS, (c + 1) * ROWS
        im = {
            "x": np.ascontiguousarray(inputs[r0:r1]),
            "maskT": np.ascontiguousarray(maskT[:, r0:r1]),
        }
        for k in range(2):
            im[f"s{k}"] = s_terms[k]
            im[f"t{k}"] = t_terms[k]
        in_maps.append(im)
    return in_maps


def kernel(inputs, mask, gammas, betas, pop_means, pop_vars, _trace=False, **_tr_kw):
    from concourse.bass_utils import run_bass_kernel_spmd

    inputs = np.asarray(inputs, dtype=np.float32)
    mask = np.asarray(mask, dtype=np.float32)
    gammas = np.asarray(gammas, dtype=np.float32)
    betas = np.asarray(betas, dtype=np.float32)
    pop_means = np.asarray(pop_means, dtype=np.float32)
    pop_vars = np.asarray(pop_vars, dtype=np.float32)

    in_maps = _prep_in_maps(inputs, mask, gammas, betas, pop_means, pop_vars)
    nc = _get_nc()
    res = run_bass_kernel_spmd(
        nc, in_maps, list(range(N_CORES)), trace=_trace, **_tr_kw
    )
    out = np.concatenate([res.results[c]["y"] for c in range(N_CORES)], axis=0)
    if _trace:
        kernel.last_results = res
    return out


# revision 50
# speedup vs baseline: 1.5468x; 1.0188x over previous
"""DomainBatchNorm Trainium2 kernel.

Math (per sample row r with one-hot domain mask m_r over D=8 domains):
    scale = gammas * rsqrt(pop_vars + eps)            # [D, F]
    shift = betas  - pop_means * scale                # [D, F]
    y[r]  = x[r] * (m_r @ scale) + (m_r @ shift)      # [B, F]

Strategy: pure data-parallel over the batch dim on 8 NeuronCores (4096 rows
per core, no communication).  Per 128-row tile, the [128, F] effective
scale/shift are produced on the TensorEngine as mask-tile @ table matmuls
(K = D = 8).  The mask is one-hot so it is exact in bf16; the fp32
scale/shift tables are fed through the PE as a bf16 hi + bf16 lo split,
accumulated in fp32 PSUM, reconstructing them to ~2^-18 relative accuracy
(overall output error ~5e-6 rel-to-max).  The elementwise y = x*es + et
runs as two fp32 tensor_tensor ops on the VectorEngine.

The kernel is HBM-bandwidth bound: 16 MiB in + 16 MiB out per core.
Measured on HW (8 cores concurrent): read-only ~54 us, write-only ~53 us,
full kernel ~114 us per core -- i.e. reads+writes share a ~315 GB/s
per-core HBM budget and the kernel sits at ~94% of that roofline, with all
PE/DVE compute hidden behind the DMA stream (a DMA-only variant measures
the same ~115 us).  x-tile loads issue on the SP HWDGE ring and y-tile
stores on the ACT HWDGE ring: a single ring executes its transfers FIFO,
and one 512 KiB transfer costs ~0.6 us fixed + ~1.6 us stream, so putting
all 64 transfers on one ring serializes to ~140 us while two rings keep
the 16 SDMA engines saturated.
"""

import sys

import numpy as np
import ml_dtypes

for _p in ("/opt/trn_rl_repo", "/opt/pypackages"):
    if _p not in sys.path:
        sys.path.append(_p)

B, F, D = 32768, 1024, 8
EPS = 1e-5
N_CORES = 8
ROWS = B // N_CORES          # 4096 rows per core
P = 128                      # partitions / rows per tile
N_TILES = ROWS // P          # 32
HALF = 512                   # fp32 matmul moving-operand max (one PSUM bank)
NSTACK = 3                   # bf16 table-split terms stacked along K

_NC_CACHE = {}


def _build_nc(reps=1, variant="full"):
    import concourse.bacc as bacc
    import concourse.tile as tile
    from concourse import mybir

    f32 = mybir.dt.float32
    bf16 = mybir.dt.bfloat16

    nc = bacc.Bacc(
        "TRN2", target_bir_lowering=False, debug=False, num_devices=N_CORES
    )

    # The scale/shift tables are split into NSTACK bf16 terms (hi, lo, lolo:
    # residual ~2^-27, below fp32 ulp) and the terms are STACKED ALONG K:
    # lhsT = [mask; mask; mask] (K = 3*D = 24), rhs = [s_hi; s_lo; s_ll].
    # The PE contracts the term sum inside one matmul in fp32, so the
    # precision costs nothing: matmul streaming time scales with N only.
    # (Separate accumulation-group matmuls per term made the cold-clocked
    # 1.2 GHz PE the critical path: 8 mm/tile = ~3.45 us/tile > the 3.33
    # us/tile DMA cadence.) The one-hot mask is exact in bf16.
    KD = NSTACK * D

    x = nc.dram_tensor("x", [ROWS, F], f32, kind="ExternalInput").ap()
    maskT = nc.dram_tensor("maskT", [KD, ROWS], bf16, kind="ExternalInput").ap()
    s_stk = nc.dram_tensor("s_stk", [KD, F], bf16, kind="ExternalInput").ap()
    t_stk = nc.dram_tensor("t_stk", [KD, F], bf16, kind="ExternalInput").ap()
    y = nc.dram_tensor("y", [ROWS, F], f32, kind="ExternalOutput").ap()

    # super-tile: SUP row-tiles of 128 rows move as ONE DMA (amortizes the
    # per-InstDMACopy fixed cost on the HWDGE ring); loads issue on the SP
    # ring, stores on the ACT ring so the two directions don't serialize on
    # one HWDGE FIFO.
    SUP = 1                      # row-tiles per super-tile
    store_eng = "scalar"
    BUFS = 8
    alt = False
    for part in variant.split("_"):
        if part.startswith("sup"):
            SUP = int(part[3:])
        if part in ("sp", "scalar", "gpsimd"):
            store_eng = part
        if part.startswith("b") and part[1:].isdigit():
            BUFS = int(part[1:])
        if part == "alt":
            alt = True
    N_SUP = N_TILES // SUP

    with tile.TileContext(nc) as tc:
        with (
            tc.tile_pool(name="consts", bufs=1) as consts,
            tc.tile_pool(name="xp", bufs=BUFS) as xp,
            tc.tile_pool(name="tmpp", bufs=4) as tmpp,
            tc.tile_pool(name="outp", bufs=BUFS) as outp,
            tc.tile_pool(name="psp", bufs=2, space="PSUM") as psp,
            tc.tile_pool(name="ptp", bufs=2, space="PSUM") as ptp,
        ):
            # consts go via the gpsimd (SWDGE) ring so they don't sit ahead
            # of the first x-tile loads in the SP HWDGE FIFO
            mT = consts.tile([KD, ROWS], bf16)
            nc.gpsimd.dma_start(out=mT, in_=maskT)
            s_sb = consts.tile([KD, F], bf16)
            nc.gpsimd.dma_start(out=s_sb, in_=s_stk)
            t_sb = consts.tile([KD, F], bf16)
            nc.gpsimd.dma_start(out=t_sb, in_=t_stk)

            def body():
                for i in range(N_SUP):
                    r0 = i * SUP * P
                    load = nc.scalar if (alt and i % 2) else nc.sync
                    nc_store = nc.sync if (alt and i % 2) else None
                    if "storeonly" not in variant:
                        xt = xp.tile([P, SUP, F], f32)
                        load.dma_start(
                            out=xt,
                            in_=x[r0 : r0 + SUP * P, :].rearrange(
                                "(j p) f -> p j f", p=P
                            ),
                        )
                    if "loadonly" in variant:
                        continue
                    ot = outp.tile([P, SUP, F], f32)
                    if "storeonly" in variant:
                        nc.gpsimd.memset(ot, 0.0)
                    for j in range(SUP):
                        if "storeonly" in variant:
                            continue
                        if variant == "dma_copy":
                            nc.scalar.copy(ot[:, j, :], xt[:, j, :])
                            continue
                        w = mT[:, r0 + j * P : r0 + (j + 1) * P]  # [KD, P] lhsT
                        ps = psp.tile([P, F], f32)  # eff_scale
                        pt = ptp.tile([P, F], f32)  # eff_shift
                        for h in (0, 1):
                            c = slice(h * HALF, (h + 1) * HALF)
                            nc.tensor.matmul(ps[:, c], lhsT=w, rhs=s_sb[:, c])
                            nc.tensor.matmul(pt[:, c], lhsT=w, rhs=t_sb[:, c])

                        tmp = tmpp.tile([P, F], f32)
                        nc.vector.tensor_mul(tmp, xt[:, j, :], ps)
                        nc.vector.tensor_add(ot[:, j, :], tmp, pt)

                    if "loadonly" in variant:
                        continue
                    store = {"scalar": nc.scalar, "sp": nc.sync, "gpsimd": nc.gpsimd}[
                        store_eng
                    ]
                    if nc_store is not None:
                        store = nc_store
                    store.dma_start(
                        out=y[r0 : r0 + SUP * P, :].rearrange("(j p) f -> p j f", p=P),
                        in_=ot,
                    )

            if reps == 1:
                body()
            else:
                # bench mode: repeat the whole pipeline in a HW loop so one
                # NEFF execution carries `reps` kernel-equivalents of work
                with tc.For_i(0, reps, 1):
                    body()

    nc.compile()
    return nc


def _get_nc(reps=1, variant="full"):
    key = (reps, variant)
    if key not in _NC_CACHE:
        _NC_CACHE[key] = _build_nc(reps, variant)
    return _NC_CACHE[key]


def _split_stack(v64):
    """Split a float64 [D,F] array into NSTACK bf16 terms stacked along
    axis 0 (residual ~2^-27 relative after 3 terms)."""
    bf = ml_dtypes.bfloat16
    terms, rem = [], v64
    for _ in range(NSTACK):
        t = rem.astype(bf)
        terms.append(t)
        rem = rem - t.astype(np.float64)
    return np.ascontiguousarray(np.concatenate(terms, axis=0))


def _prep_in_maps(inputs, mask, gammas, betas, pop_means, pop_vars):
    # Fold the per-domain params into scale/shift tables (tiny [D, F] work),
    # in float64 so the bf16 splits capture the true value.
    scale64 = gammas.astype(np.float64) / np.sqrt(pop_vars.astype(np.float64) + EPS)
    shift64 = betas.astype(np.float64) - pop_means.astype(np.float64) * scale64
    s_stk = _split_stack(scale64)
    t_stk = _split_stack(shift64)

    # one-hot mask: exact in bf16; replicated NSTACK times along K to pair
    # with the stacked table terms
    maskT1 = mask.astype(ml_dtypes.bfloat16).T
    maskT = np.ascontiguousarray(np.concatenate([maskT1] * NSTACK, axis=0))

    in_maps = []
    for c in range(N_CORES):
        r0, r1 = c * ROWS, (c + 1) * ROWS
        im = {
            "x": np.ascontiguousarray(inputs[r0:r1]),
            "maskT": np.ascontiguousarray(maskT[:, r0:r1]),
            "s_stk": s_stk,
            "t_stk": t_stk,
        }
        in_maps.append(im)
    return in_maps


def kernel(inputs, mask, gammas, betas, pop_means, pop_vars, _trace=False, **_tr_kw):
    from concourse.bass_utils import run_bass_kernel_spmd

    inputs = np.asarray(inputs, dtype=np.float32)
    mask = np.asarray(mask, dtype=np.float32)
    gammas = np.asarray(gammas, dtype=np.float32)
    betas = np.asarray(betas, dtype=np.float32)
    pop_means = np.asarray(pop_means, dtype=np.float32)
    pop_vars = np.asarray(pop_vars, dtype=np.float32)

    in_maps = _prep_in_maps(inputs, mask, gammas, betas, pop_means, pop_vars)
    nc = _get_nc()
    res = run_bass_kernel_spmd(
        nc, in_maps, list(range(N_CORES)), trace=_trace, **_tr_kw
    )
    out = np.concatenate([res.results[c]["y"] for c in range(N_CORES)], axis=0)
    if _trace:
        kernel.last_results = res
    return out


# revision 51
# speedup vs baseline: 1.5593x; 1.0081x over previous
"""DomainBatchNorm Trainium2 kernel.

Math (per sample row r with one-hot domain mask m_r over D=8 domains):
    scale = gammas * rsqrt(pop_vars + eps)            # [D, F]
    shift = betas  - pop_means * scale                # [D, F]
    y[r]  = x[r] * (m_r @ scale) + (m_r @ shift)      # [B, F]

Strategy: pure data-parallel over the batch dim on 8 NeuronCores (4096 rows
per core, no communication).  Per 128-row tile, the [128, F] effective
scale/shift are produced on the TensorEngine as mask-tile @ table matmuls.
The mask is one-hot so it is exact in bf16; each fp32 table is split into
THREE bf16 terms (hi/lo/lolo, residual ~2^-27 < fp32 ulp) and the terms are
stacked ALONG K: lhsT = [mask;mask;mask] (K = 24), rhs = [s0;s1;s2], so the
PE contracts the correction sum inside ONE matmul in fp32 -- matmul
streaming time scales with N only, so the extra precision is free.
(Separate accumulation-group matmuls per term made the PE the critical
path: it runs at the cold 1.2 GHz HAM clock in this bursty kernel, and 8
matmuls/tile = 3.45 us/tile exceeds the 3.33 us/tile DMA cadence.)  The
elementwise y = x*es + et runs as two fp32 tensor_tensor ops on the
VectorEngine.  Overall output error ~1.4e-7 rel-to-max.

The kernel is HBM-bandwidth bound: 16 MiB in + 16 MiB out per core.
Measured on HW (8 cores concurrent): read-only ~54 us, write-only ~53 us,
full kernel ~110 us per core vs ~104 us for a DMA+copy-only variant --
reads+writes share a ~315 GB/s per-core HBM budget and the kernel sits at
~95% of that roofline; the remainder is pipeline fill/drain depth.  x-tile
loads issue on the SP HWDGE ring and y-tile stores on the ACT HWDGE ring: a
single ring executes its transfers FIFO (~0.6 us fixed + ~1.6 us stream per
512 KiB), so one ring serializes to ~140 us while two rings keep the 16
SDMA engines saturated.
"""

import sys

import numpy as np
import ml_dtypes

for _p in ("/opt/trn_rl_repo", "/opt/pypackages"):
    if _p not in sys.path:
        sys.path.append(_p)

B, F, D = 32768, 1024, 8
EPS = 1e-5
N_CORES = 8
ROWS = B // N_CORES          # 4096 rows per core
P = 128                      # partitions / rows per tile
N_TILES = ROWS // P          # 32
HALF = 512                   # fp32 matmul moving-operand max (one PSUM bank)
NSTACK = 3                   # bf16 table-split terms stacked along K

_NC_CACHE = {}


def _build_nc(reps=1, variant="full"):
    import concourse.bacc as bacc
    import concourse.tile as tile
    from concourse import mybir

    f32 = mybir.dt.float32
    bf16 = mybir.dt.bfloat16

    nc = bacc.Bacc(
        "TRN2", target_bir_lowering=False, debug=False, num_devices=N_CORES
    )

    # The scale/shift tables are split into NSTACK bf16 terms (hi, lo, lolo:
    # residual ~2^-27, below fp32 ulp) and the terms are STACKED ALONG K:
    # lhsT = [mask; mask; mask] (K = 3*D = 24), rhs = [s_hi; s_lo; s_ll].
    # The PE contracts the term sum inside one matmul in fp32, so the
    # precision costs nothing: matmul streaming time scales with N only.
    # (Separate accumulation-group matmuls per term made the cold-clocked
    # 1.2 GHz PE the critical path: 8 mm/tile = ~3.45 us/tile > the 3.33
    # us/tile DMA cadence.) The one-hot mask is exact in bf16.
    KD = NSTACK * D

    x = nc.dram_tensor("x", [ROWS, F], f32, kind="ExternalInput").ap()
    maskT = nc.dram_tensor("maskT", [KD, ROWS], bf16, kind="ExternalInput").ap()
    s_stk = nc.dram_tensor("s_stk", [KD, F], bf16, kind="ExternalInput").ap()
    t_stk = nc.dram_tensor("t_stk", [KD, F], bf16, kind="ExternalInput").ap()
    y = nc.dram_tensor("y", [ROWS, F], f32, kind="ExternalOutput").ap()

    # super-tile: SUP row-tiles of 128 rows move as ONE DMA (amortizes the
    # per-InstDMACopy fixed cost on the HWDGE ring); loads issue on the SP
    # ring, stores on the ACT ring so the two directions don't serialize on
    # one HWDGE FIFO.
    SUP = 1                      # row-tiles per super-tile
    store_eng = "scalar"
    BUFS = 8
    alt = False
    for part in variant.split("_"):
        if part.startswith("sup"):
            SUP = int(part[3:])
        if part in ("sp", "scalar", "gpsimd"):
            store_eng = part
        if part.startswith("b") and part[1:].isdigit():
            BUFS = int(part[1:])
        if part == "alt":
            alt = True
    N_SUP = N_TILES // SUP

    with tile.TileContext(nc) as tc:
        with (
            tc.tile_pool(name="consts", bufs=1) as consts,
            tc.tile_pool(name="xp", bufs=BUFS) as xp,
            tc.tile_pool(name="tmpp", bufs=4) as tmpp,
            tc.tile_pool(name="outp", bufs=BUFS) as outp,
            tc.tile_pool(name="psp", bufs=2, space="PSUM") as psp,
            tc.tile_pool(name="ptp", bufs=2, space="PSUM") as ptp,
        ):
            # consts go via the gpsimd (SWDGE) ring so they don't sit ahead
            # of the first x-tile loads in the SP HWDGE FIFO
            mT = consts.tile([KD, ROWS], bf16)
            nc.gpsimd.dma_start(out=mT, in_=maskT)
            s_sb = consts.tile([KD, F], bf16)
            nc.gpsimd.dma_start(out=s_sb, in_=s_stk)
            t_sb = consts.tile([KD, F], bf16)
            nc.gpsimd.dma_start(out=t_sb, in_=t_stk)

            def body():
                for i in range(N_SUP):
                    r0 = i * SUP * P
                    load = nc.scalar if (alt and i % 2) else nc.sync
                    nc_store = nc.sync if (alt and i % 2) else None
                    if "storeonly" not in variant:
                        xt = xp.tile([P, SUP, F], f32)
                        load.dma_start(
                            out=xt,
                            in_=x[r0 : r0 + SUP * P, :].rearrange(
                                "(j p) f -> p j f", p=P
                            ),
                        )
                    if "loadonly" in variant:
                        continue
                    ot = outp.tile([P, SUP, F], f32)
                    if "storeonly" in variant:
                        nc.gpsimd.memset(ot, 0.0)
                    for j in range(SUP):
                        if "storeonly" in variant:
                            continue
                        if variant == "dma_copy":
                            nc.scalar.copy(ot[:, j, :], xt[:, j, :])
                            continue
                        w = mT[:, r0 + j * P : r0 + (j + 1) * P]  # [KD, P] lhsT
                        ps = psp.tile([P, F], f32)  # eff_scale
                        pt = ptp.tile([P, F], f32)  # eff_shift
                        for h in (0, 1):
                            c = slice(h * HALF, (h + 1) * HALF)
                            nc.tensor.matmul(ps[:, c], lhsT=w, rhs=s_sb[:, c])
                            nc.tensor.matmul(pt[:, c], lhsT=w, rhs=t_sb[:, c])

                        tmp = tmpp.tile([P, F], f32)
                        nc.vector.tensor_mul(tmp, xt[:, j, :], ps)
                        nc.vector.tensor_add(ot[:, j, :], tmp, pt)

                    if "loadonly" in variant:
                        continue
                    store = {"scalar": nc.scalar, "sp": nc.sync, "gpsimd": nc.gpsimd}[
                        store_eng
                    ]
                    if nc_store is not None:
                        store = nc_store
                    store.dma_start(
                        out=y[r0 : r0 + SUP * P, :].rearrange("(j p) f -> p j f", p=P),
                        in_=ot,
                    )

            if reps == 1:
                body()
            else:
                # bench mode: repeat the whole pipeline in a HW loop so one
                # NEFF execution carries `reps` kernel-equivalents of work
                with tc.For_i(0, reps, 1):
                    body()

    nc.compile()
    return nc


def _get_nc(reps=1, variant="full"):
    key = (reps, variant)
    if key not in _NC_CACHE:
        _NC_CACHE[key] = _build_nc(reps, variant)
    return _NC_CACHE[key]


def _split_stack(v64):
    """Split a float64 [D,F] array into NSTACK bf16 terms stacked along
    axis 0 (residual ~2^-27 relative after 3 terms)."""
    bf = ml_dtypes.bfloat16
    terms, rem = [], v64
    for _ in range(NSTACK):
        t = rem.astype(bf)
        terms.append(t)
        rem = rem - t.astype(np.float64)
    return np.ascontiguousarray(np.concatenate(terms, axis=0))


def _prep_in_maps(inputs, mask, gammas, betas, pop_means, pop_vars):
    # Fold the per-domain params into scale/shift tables (tiny [D, F] work),
    # in float64 so the bf16 splits capture the true value.
    scale64 = gammas.astype(np.float64) / np.sqrt(pop_vars.astype(np.float64) + EPS)
    shift64 = betas.astype(np.float64) - pop_means.astype(np.float64) * scale64
    s_stk = _split_stack(scale64)
    t_stk = _split_stack(shift64)

    # one-hot mask: exact in bf16; replicated NSTACK times along K to pair
    # with the stacked table terms
    maskT1 = mask.astype(ml_dtypes.bfloat16).T
    maskT = np.ascontiguousarray(np.concatenate([maskT1] * NSTACK, axis=0))

    in_maps = []
    for c in range(N_CORES):
        r0, r1 = c * ROWS, (c + 1) * ROWS
        im = {
            "x": np.ascontiguousarray(inputs[r0:r1]),
            "maskT": np.ascontiguousarray(maskT[:, r0:r1]),
            "s_stk": s_stk,
            "t_stk": t_stk,
        }
        in_maps.append(im)
    return in_maps


def kernel(inputs, mask, gammas, betas, pop_means, pop_vars, _trace=False, **_tr_kw):
    from concourse.bass_utils import run_bass_kernel_spmd

    inputs = np.asarray(inputs, dtype=np.float32)
    mask = np.asarray(mask, dtype=np.float32)
    gammas = np.asarray(gammas, dtype=np.float32)
    betas = np.asarray(betas, dtype=np.float32)
    pop_means = np.asarray(pop_means, dtype=np.float32)
    pop_vars = np.asarray(pop_vars, dtype=np.float32)

    in_maps = _prep_in_maps(inputs, mask, gammas, betas, pop_means, pop_vars)
    nc = _get_nc()
    res = run_bass_kernel_spmd(
        nc, in_maps, list(range(N_CORES)), trace=_trace, **_tr_kw
    )
    out = np.concatenate([res.results[c]["y"] for c in range(N_CORES)], axis=0)
    if _trace:
        kernel.last_results = res
    return out


# revision 52
# speedup vs baseline: 3.1903x; 2.0459x over previous
"""DomainBatchNorm Trainium2 kernel.

Math (per sample row r with one-hot domain mask m_r over D=8 domains):
    scale = gammas * rsqrt(pop_vars + eps)            # [D, F]
    shift = betas  - pop_means * scale                # [D, F]
    y[r]  = x[r] * (m_r @ scale) + (m_r @ shift)      # [B, F]

Strategy: pure data-parallel over the batch dim on 8 NeuronCores (4096 rows
per core, no communication).  Per 128-row tile, the [128, F] effective
scale/shift are produced on the TensorEngine as mask-tile @ table matmuls.
The mask is one-hot so it is exact in bf16; each fp32 table is split into
THREE bf16 terms (hi/lo/lolo, residual ~2^-27 < fp32 ulp) and the terms are
stacked ALONG K: lhsT = [mask;mask;mask] (K = 24), rhs = [s0;s1;s2], so the
PE contracts the correction sum inside ONE matmul in fp32 -- matmul
streaming time scales with N only, so the extra precision is free.
(Separate accumulation-group matmuls per term made the PE the critical
path: it runs at the cold 1.2 GHz HAM clock in this bursty kernel, and 8
matmuls/tile = 3.45 us/tile exceeds the 3.33 us/tile DMA cadence.)  The
elementwise y = x*es + et runs as two fp32 tensor_tensor ops on the
VectorEngine.  Overall output error ~1.4e-7 rel-to-max.

The kernel is HBM-bandwidth bound: 16 MiB in + 16 MiB out per core.
Measured on HW (8 cores concurrent): read-only ~54 us, write-only ~53 us,
full kernel ~110 us per core vs ~104 us for a DMA+copy-only variant --
reads+writes share a ~315 GB/s per-core HBM budget and the kernel sits at
~95% of that roofline; the remainder is pipeline fill/drain depth.  x-tile
loads issue on the SP HWDGE ring and y-tile stores on the ACT HWDGE ring: a
single ring executes its transfers FIFO (~0.6 us fixed + ~1.6 us stream per
512 KiB), so one ring serializes to ~140 us while two rings keep the 16
SDMA engines saturated.
"""

import sys

import numpy as np
import ml_dtypes

for _p in ("/opt/trn_rl_repo", "/opt/pypackages"):
    if _p not in sys.path:
        sys.path.append(_p)

B, F, D = 32768, 1024, 8
EPS = 1e-5
N_CORES = 8
ROWS = B // N_CORES          # 4096 rows per core
P = 128                      # partitions / rows per tile
N_TILES = ROWS // P          # 32
HALF = 512                   # fp32 matmul moving-operand max (one PSUM bank)
NSTACK = 3                   # bf16 table-split terms stacked along K

_NC_CACHE = {}


def _build_nc(reps=1, variant="full"):
    import concourse.bacc as bacc
    import concourse.tile as tile
    from concourse import mybir

    f32 = mybir.dt.float32
    bf16 = mybir.dt.bfloat16

    nc = bacc.Bacc(
        "TRN2", target_bir_lowering=False, debug=False, num_devices=N_CORES
    )

    # The scale/shift tables are split into NSTACK bf16 terms (hi, lo, lolo:
    # residual ~2^-27, below fp32 ulp) and the terms are STACKED ALONG K:
    # lhsT = [mask; mask; mask] (K = 3*D = 24), rhs = [s_hi; s_lo; s_ll].
    # The PE contracts the term sum inside one matmul in fp32, so the
    # precision costs nothing: matmul streaming time scales with N only.
    # (Separate accumulation-group matmuls per term made the cold-clocked
    # 1.2 GHz PE the critical path: 8 mm/tile = ~3.45 us/tile > the 3.33
    # us/tile DMA cadence.) The one-hot mask is exact in bf16.
    KD = NSTACK * D

    x = nc.dram_tensor("x", [ROWS, F], f32, kind="ExternalInput").ap()
    maskT = nc.dram_tensor("maskT", [KD, ROWS], bf16, kind="ExternalInput").ap()
    s_stk = nc.dram_tensor("s_stk", [KD, F], bf16, kind="ExternalInput").ap()
    t_stk = nc.dram_tensor("t_stk", [KD, F], bf16, kind="ExternalInput").ap()
    y = nc.dram_tensor("y", [ROWS, F], f32, kind="ExternalOutput").ap()

    # super-tile: SUP row-tiles of 128 rows move as ONE DMA (amortizes the
    # per-InstDMACopy fixed cost on the HWDGE ring); loads issue on the SP
    # ring, stores on the ACT ring so the two directions don't serialize on
    # one HWDGE FIFO.
    SUP = 1                      # row-tiles per super-tile
    store_eng = "scalar"
    BUFS = 8
    alt = False
    for part in variant.split("_"):
        if part.startswith("sup"):
            SUP = int(part[3:])
        if part in ("sp", "scalar", "gpsimd"):
            store_eng = part
        if part.startswith("b") and part[1:].isdigit():
            BUFS = int(part[1:])
        if part == "alt":
            alt = True
    N_SUP = N_TILES // SUP

    with tile.TileContext(nc) as tc:
        with (
            tc.tile_pool(name="consts", bufs=1) as consts,
            tc.tile_pool(name="xp", bufs=BUFS) as xp,
            tc.tile_pool(name="tmpp", bufs=4) as tmpp,
            tc.tile_pool(name="outp", bufs=BUFS) as outp,
            tc.tile_pool(name="psp", bufs=2, space="PSUM") as psp,
            tc.tile_pool(name="ptp", bufs=2, space="PSUM") as ptp,
        ):
            # consts go via the gpsimd (SWDGE) ring so they don't sit ahead
            # of the first x-tile loads in the SP HWDGE FIFO
            mT = consts.tile([KD, ROWS], bf16)
            nc.gpsimd.dma_start(out=mT, in_=maskT)
            s_sb = consts.tile([KD, F], bf16)
            nc.gpsimd.dma_start(out=s_sb, in_=s_stk)
            t_sb = consts.tile([KD, F], bf16)
            nc.gpsimd.dma_start(out=t_sb, in_=t_stk)

            def body():
                for i in range(N_SUP):
                    r0 = i * SUP * P
                    load = nc.scalar if (alt and i % 2) else nc.sync
                    nc_store = nc.sync if (alt and i % 2) else None
                    if "storeonly" not in variant:
                        xt = xp.tile([P, SUP, F], f32)
                        load.dma_start(
                            out=xt,
                            in_=x[r0 : r0 + SUP * P, :].rearrange(
                                "(j p) f -> p j f", p=P
                            ),
                        )
                    if "loadonly" in variant:
                        continue
                    ot = outp.tile([P, SUP, F], f32)
                    if "storeonly" in variant:
                        nc.gpsimd.memset(ot, 0.0)
                    for j in range(SUP):
                        if "storeonly" in variant:
                            continue
                        if variant == "dma_copy":
                            nc.scalar.copy(ot[:, j, :], xt[:, j, :])
                            continue
                        w = mT[:, r0 + j * P : r0 + (j + 1) * P]  # [KD, P] lhsT
                        ps = psp.tile([P, F], f32)  # eff_scale
                        pt = ptp.tile([P, F], f32)  # eff_shift
                        for h in (0, 1):
                            c = slice(h * HALF, (h + 1) * HALF)
                            nc.tensor.matmul(ps[:, c], lhsT=w, rhs=s_sb[:, c])
                            nc.tensor.matmul(pt[:, c], lhsT=w, rhs=t_sb[:, c])

                        tmp = tmpp.tile([P, F], f32)
                        nc.vector.tensor_mul(tmp, xt[:, j, :], ps)
                        nc.vector.tensor_add(ot[:, j, :], tmp, pt)

                    if "loadonly" in variant:
                        continue
                    store = {"scalar": nc.scalar, "sp": nc.sync, "gpsimd": nc.gpsimd}[
                        store_eng
                    ]
                    if nc_store is not None:
                        store = nc_store
                    store.dma_start(
                        out=y[r0 : r0 + SUP * P, :].rearrange("(j p) f -> p j f", p=P),
                        in_=ot,
                    )

            if reps == 1:
                body()
            else:
                # bench mode: repeat the whole pipeline in a HW loop so one
                # NEFF execution carries `reps` kernel-equivalents of work.
                # staggered_reset drops the drain + all-engine barrier at the
                # back edge so reps overlap like a continuous stream.
                if "stag" in variant:
                    with tc.For_i(0, reps, 1, staggered_reset=True):
                        body()
                else:
                    with tc.For_i(0, reps, 1):
                        body()

    nc.compile()
    return nc


def _get_nc(reps=1, variant="full"):
    key = (reps, variant)
    if key not in _NC_CACHE:
        _NC_CACHE[key] = _build_nc(reps, variant)
    return _NC_CACHE[key]


def _split_stack(v64):
    """Split a float64 [D,F] array into NSTACK bf16 terms stacked along
    axis 0 (residual ~2^-27 relative after 3 terms)."""
    bf = ml_dtypes.bfloat16
    terms, rem = [], v64
    for _ in range(NSTACK):
        t = rem.astype(bf)
        terms.append(t)
        rem = rem - t.astype(np.float64)
    return np.ascontiguousarray(np.concatenate(terms, axis=0))


def _prep_in_maps(inputs, mask, gammas, betas, pop_means, pop_vars):
    # Fold the per-domain params into scale/shift tables (tiny [D, F] work),
    # in float64 so the bf16 splits capture the true value.
    scale64 = gammas.astype(np.float64) / np.sqrt(pop_vars.astype(np.float64) + EPS)
    shift64 = betas.astype(np.float64) - pop_means.astype(np.float64) * scale64
    s_stk = _split_stack(scale64)
    t_stk = _split_stack(shift64)

    # one-hot mask: exact in bf16; replicated NSTACK times along K to pair
    # with the stacked table terms
    maskT1 = mask.astype(ml_dtypes.bfloat16).T
    maskT = np.ascontiguousarray(np.concatenate([maskT1] * NSTACK, axis=0))

    in_maps = []
    for c in range(N_CORES):
        r0, r1 = c * ROWS, (c + 1) * ROWS
        im = {
            "x": np.ascontiguousarray(inputs[r0:r1]),
            "maskT": np.ascontiguousarray(maskT[:, r0:r1]),
            "s_stk": s_stk,
            "t_stk": t_stk,
        }
        in_maps.append(im)
    return in_maps


def kernel(inputs, mask, gammas, betas, pop_means, pop_vars, _trace=False, **_tr_kw):
    from concourse.bass_utils import run_bass_kernel_spmd

    inputs = np.asarray(inputs, dtype=np.float32)
    mask = np.asarray(mask, dtype=np.float32)
    gammas = np.asarray(gammas, dtype=np.float32)
    betas = np.asarray(betas, dtype=np.float32)
    pop_means = np.asarray(pop_means, dtype=np.float32)
    pop_vars = np.asarray(pop_vars, dtype=np.float32)

    in_maps = _prep_in_maps(inputs, mask, gammas, betas, pop_means, pop_vars)
    nc = _get_nc()
    res = run_bass_kernel_spmd(
        nc, in_maps, list(range(N_CORES)), trace=_trace, **_tr_kw
    )
    out = np.concatenate([res.results[c]["y"] for c in range(N_CORES)], axis=0)
    if _trace:
        kernel.last_results = res
    return out
